# revision 1
# baseline (speedup 1.0000x reference)
"""Trainium2 Bass kernel for nn_Encoder_70781061038947.

Math: row b's output depends on x[b, :] only through its 16 sign bits
(root k has radius R if x[b,k] > 0 else 1/R, phase shuffle_vector[k]).
The monic degree-16 polynomial is a product of three sub-polynomials over
bit-groups (6+5+5 bits).  Evaluate each group's sub-polynomial at the 17th
roots of unity via a one-hot matmul against a tiny table (64/32/32 rows),
multiply the three evaluations pointwise per row, normalize via Parseval
(||coeffs||^2 = mean |P(t_m)|^2), and interpolate coefficients back with a
17-point inverse-DFT matmul.  All O(B) work runs on-device:

  PE : sign transposes, one-hot match matmuls (bf16), table-gather matmuls
       (split-precision bf16 hi+lo), eval transposes, inverse-DFT matmuls
  ACT: sign extraction, one-hot thresholding relu(count + bias), eval
       staging copy, sqrt for the norm factor
  DVE/GPSIMD: pointwise complex products, norm, PSUM->SBUF moves

Sharding: pure data parallel over B across 8 cores (32768 rows each); the
small tables derived from shuffle_vector (host FLOPs independent of B) are
replicated inputs.
"""

import numpy as np
import ml_dtypes

import concourse.bacc as bacc
import concourse.bass as bass
import concourse.mybir as mybir
import concourse.bass_utils as bass_utils
import concourse.tile as tile

B = 262144
K = 16
M = 17                      # evaluation points (17th roots of unity)
W = 2 * M                   # 34 f32 per output row
NCORES = 8
RPC = B // NCORES           # 32768 rows per core
P = 128
CPB = RPC // P              # 256 rows per partition
TPC = 8                     # tiles (row-columns) per chunk
NCHUNK = CPB // TPC         # 32 chunks
GROUPS = [(0, 6), (6, 5), (11, 5)]   # (base bit, size): one-hot rows 64+32+32 = 128

_cached = None


def _tables(shuffle_vector: np.ndarray):
    sv = np.asarray(shuffle_vector, dtype=np.float64)
    R = np.sqrt(1.0 + np.sin(np.pi / K))
    t = np.exp(2j * np.pi * np.arange(M) / M)
    bf16 = ml_dtypes.bfloat16

    tbl = np.zeros((P, 3 * W), np.float64)   # [(g,nu), 34g + re/im]
    w3 = np.zeros((K, P), np.float64)
    biasv = np.zeros((P, 1), np.float64)
    row = 0
    for g, (base, size) in enumerate(GROUPS):
        for nu in range(1 << size):
            E = np.ones(M, np.complex128)
            for j in range(size):
                b = (nu >> j) & 1
                zk = (R if b else 1.0 / R) * np.exp(1j * sv[base + j])
                E = E * (t - zk)
            tbl[row, W * g: W * g + M] = E.real
            tbl[row, W * g + M: W * g + W] = E.imag
            for j in range(size):
                w3[base + j, row] = 2.0 * ((nu >> j) & 1) - 1.0
            # signs are +-1 on device: full match <=> dot == size
            biasv[row, 0] = 1 - size
            row += 1
    assert row == P

    # split-precision eval table: tbl ~= hi + lo with both halves bf16
    tbl_hi = tbl.astype(bf16)
    tbl_lo = (tbl - tbl_hi.astype(np.float64)).astype(bf16)

    w2r = np.zeros((W, W), np.float64)       # [re17||im17, interleaved re/im out]
    for m in range(M):
        for d in range(M):
            w = np.exp(-2j * np.pi * ((K - d) * m) / M) / M
            w2r[m, 2 * d] = w.real
            w2r[m, 2 * d + 1] = w.imag
            w2r[M + m, 2 * d] = -w.imag
            w2r[M + m, 2 * d + 1] = w.real

    # block-diagonal variants: 3-tile (102x102) and 2-tile (68x68) groups
    w2r3 = np.zeros((3 * W, 3 * W), np.float64)
    for j in range(3):
        w2r3[j * W:(j + 1) * W, j * W:(j + 1) * W] = w2r
    w2r2 = np.zeros((2 * W, 2 * W), np.float64)
    for j in range(2):
        w2r2[j * W:(j + 1) * W, j * W:(j + 1) * W] = w2r

    ident_bf = np.eye(P, dtype=bf16)
    ident_f = np.eye(P, dtype=np.float32)

    return {
        "w3": w3.astype(bf16),
        "biasv": biasv.astype(np.float32),
        "tblhi": tbl_hi,
        "tbllo": tbl_lo,
        "w2r3": w2r3.astype(np.float32),
        "w2r2": w2r2.astype(np.float32),
        "identb": ident_bf,
        "identf": ident_f,
    }


def _build_module(rpc=RPC):
    cpb = rpc // P
    nchunk = cpb // TPC
    f32 = mybir.dt.float32
    bf = mybir.dt.bfloat16
    FT = TPC * K             # 128: free width of one chunk of x
    FO = TPC * W             # 272: free width of one chunk of out
    AF = mybir.ActivationFunctionType
    OP = mybir.AluOpType

    nc = bacc.Bacc("TRN2", target_bir_lowering=False, debug=False)
    x_d = nc.dram_tensor("x", [rpc, K], bf, kind="ExternalInput")
    w3_d = nc.dram_tensor("w3", [K, P], bf, kind="ExternalInput")
    bias_d = nc.dram_tensor("biasv", [P, 1], f32, kind="ExternalInput")
    tblhi_d = nc.dram_tensor("tblhi", [P, 3 * W], bf, kind="ExternalInput")
    tbllo_d = nc.dram_tensor("tbllo", [P, 3 * W], bf, kind="ExternalInput")
    w2r3_d = nc.dram_tensor("w2r3", [3 * W, 3 * W], f32, kind="ExternalInput")
    w2r2_d = nc.dram_tensor("w2r2", [2 * W, 2 * W], f32, kind="ExternalInput")
    identb_d = nc.dram_tensor("identb", [P, P], bf, kind="ExternalInput")
    identf_d = nc.dram_tensor("identf", [P, P], f32, kind="ExternalInput")
    out_d = nc.dram_tensor("out", [rpc, W], f32, kind="ExternalOutput")

    # row (p*cpb + c) -> partition p, column c
    x_v = x_d.ap().rearrange("(p c) k -> p (c k)", p=P)      # [128, cpb*16]
    out_v = out_d.ap().rearrange("(p c) e -> p (c e)", p=P)  # [128, cpb*34]

    with tile.TileContext(nc) as tc:
        with (
            tc.tile_pool(name="const", bufs=1) as cp,
            tc.tile_pool(name="sb", bufs=4) as sp,
            tc.tile_pool(name="ps", bufs=1, space="PSUM") as pp,
        ):
            w3_sb = cp.tile([K, P], bf)
            nc.sync.dma_start(out=w3_sb[:], in_=w3_d.ap())
            bias_sb = cp.tile([P, 1], f32)
            nc.sync.dma_start(out=bias_sb[:], in_=bias_d.ap())
            tblhi_sb = cp.tile([P, 3 * W], bf)
            nc.sync.dma_start(out=tblhi_sb[:], in_=tblhi_d.ap())
            tbllo_sb = cp.tile([P, 3 * W], bf)
            nc.sync.dma_start(out=tbllo_sb[:], in_=tbllo_d.ap())
            w2r3_sb = cp.tile([3 * W, 3 * W], f32)
            nc.sync.dma_start(out=w2r3_sb[:], in_=w2r3_d.ap())
            w2r2_sb = cp.tile([2 * W, 2 * W], f32)
            nc.sync.dma_start(out=w2r2_sb[:], in_=w2r2_d.ap())
            identb = cp.tile([P, P], bf)
            nc.sync.dma_start(out=identb[:], in_=identb_d.ap())
            identf = cp.tile([P, P], f32)
            nc.sync.dma_start(out=identf[:], in_=identf_d.ap())

            for ci in range(nchunk):
                x_sb = sp.tile([P, FT], bf, tag="x")
                nc.sync.dma_start(out=x_sb[:], in_=x_v[:, ci * FT:(ci + 1) * FT])

                # per-tile transposes into one [16, 8*128] PSUM tile, then one
                # Sign: s_big[k, t*128+p] = sign(x of tile t row p), +-1 bf16
                xT = pp.tile([K, TPC * P], bf, tag="xT", bufs=2)
                for t in range(TPC):
                    nc.tensor.transpose(
                        out=xT[:, t * P:(t + 1) * P],
                        in_=x_sb[:, t * K:(t + 1) * K],
                        identity=identb[:])
                s_big = sp.tile([K, TPC * P], bf, tag="sbig")
                nc.scalar.activation(out=s_big[:], in_=xT[:], func=AF.Sign)

                # match counts: one merged matmul pair (K=16, N=512 each)
                mt = pp.tile([P, TPC * P], f32, tag="mtvr")
                for h in range(2):
                    nc.tensor.matmul(
                        out=mt[:, h * 512:(h + 1) * 512],
                        lhsT=w3_sb[:],
                        rhs=s_big[:, h * 512:(h + 1) * 512],
                        start=True, stop=True)

                ohT = sp.tile([P, TPC * P], bf, tag="ohT")
                nc.scalar.activation(
                    out=ohT[:], in_=mt[:], func=AF.Relu, bias=bias_sb[:], scale=1.0)

                # gather: per tile, split-precision bf16 hi+lo accumulated
                vr = pp.tile([P, TPC * P], f32, tag="vr")
                for t in range(TPC):
                    nc.tensor.matmul(
                        out=vr[:, t * P: t * P + 3 * W],
                        lhsT=ohT[:, t * P:(t + 1) * P],
                        rhs=tblhi_sb[:],
                        start=True, stop=False)
                    nc.tensor.matmul(
                        out=vr[:, t * P: t * P + 3 * W],
                        lhsT=ohT[:, t * P:(t + 1) * P],
                        rhs=tbllo_sb[:],
                        start=False, stop=True)

                # stage all evals into SBUF, packed 102 per tile
                ev_sb = sp.tile([P, TPC * 3 * W], f32, tag="evsb")
                evv = ev_sb[:].rearrange("p (t e) -> p t e", e=3 * W)
                nc.scalar.activation(
                    out=evv,
                    in_=vr[:].rearrange("p (t e) -> p t e", e=P)[:, :, 0:3 * W],
                    func=AF.Copy)
                e1r, e1i = evv[:, :, 0:M], evv[:, :, M:W]
                e2r, e2i = evv[:, :, W:W + M], evv[:, :, W + M:2 * W]
                e3r, e3i = evv[:, :, 2 * W:2 * W + M], evv[:, :, 2 * W + M:3 * W]

                def mk(tag):
                    return sp.tile([P, TPC * M], f32, tag=tag, name=tag)

                t1, t2, t3, t4 = mk("t1"), mk("t2"), mk("t3"), mk("t4")
                TR, TI = mk("TR"), mk("TI")
                t1v = t1[:].rearrange("p (t e) -> p t e", e=M)
                t2v = t2[:].rearrange("p (t e) -> p t e", e=M)
                t3v = t3[:].rearrange("p (t e) -> p t e", e=M)
                t4v = t4[:].rearrange("p (t e) -> p t e", e=M)
                nc.vector.tensor_tensor(out=t1v, in0=e1r, in1=e2r, op=OP.mult)
                nc.vector.tensor_tensor(out=t2v, in0=e1i, in1=e2i, op=OP.mult)
                nc.vector.tensor_tensor(out=t3v, in0=e1r, in1=e2i, op=OP.mult)
                nc.vector.tensor_tensor(out=t4v, in0=e1i, in1=e2r, op=OP.mult)
                nc.gpsimd.tensor_tensor(out=TR[:], in0=t1[:], in1=t2[:], op=OP.subtract)
                nc.gpsimd.tensor_tensor(out=TI[:], in0=t3[:], in1=t4[:], op=OP.add)

                u1, u2, u3, u4 = mk("u1"), mk("u2"), mk("u3"), mk("u4")
                TRv = TR[:].rearrange("p (t e) -> p t e", e=M)
                TIv = TI[:].rearrange("p (t e) -> p t e", e=M)
                u1v = u1[:].rearrange("p (t e) -> p t e", e=M)
                u2v = u2[:].rearrange("p (t e) -> p t e", e=M)
                u3v = u3[:].rearrange("p (t e) -> p t e", e=M)
                u4v = u4[:].rearrange("p (t e) -> p t e", e=M)
                nc.vector.tensor_tensor(out=u1v, in0=TRv, in1=e3r, op=OP.mult)
                nc.vector.tensor_tensor(out=u2v, in0=TIv, in1=e3i, op=OP.mult)
                nc.vector.tensor_tensor(out=u3v, in0=TRv, in1=e3i, op=OP.mult)
                nc.vector.tensor_tensor(out=u4v, in0=TIv, in1=e3r, op=OP.mult)

                # VC layout [128, (t), re17||im17] packed 34 per tile
                vc = sp.tile([P, FO], f32, tag="vc")
                vcv = vc[:].rearrange("p (t e) -> p t e", e=W)
                nc.gpsimd.tensor_tensor(
                    out=vcv[:, :, 0:M], in0=u1v, in1=u2v, op=OP.subtract)
                nc.gpsimd.tensor_tensor(
                    out=vcv[:, :, M:W], in0=u3v, in1=u4v, op=OP.add)

                sq = sp.tile([P, FO], f32, tag="sq")
                sqv = sq[:].rearrange("p (t e) -> p t e", e=W)
                nc.gpsimd.tensor_tensor(out=sqv, in0=vcv, in1=vcv, op=OP.mult)
                S = sp.tile([P, TPC], f32, tag="S")
                nc.vector.tensor_reduce(
                    out=S[:], in_=sqv, axis=mybir.AxisListType.X, op=OP.add)
                rS = sp.tile([P, TPC], f32, tag="rS")
                nc.vector.reciprocal(out=rS[:], in_=S[:])
                fac = sp.tile([P, TPC], f32, tag="fac")
                nc.scalar.activation(
                    out=fac[:], in_=rS[:], func=AF.Sqrt, bias=0.0, scale=float(M * M))
                nc.vector.tensor_tensor(
                    out=vcv, in0=vcv,
                    in1=fac[:].unsqueeze(2).to_broadcast([P, TPC, W]),
                    op=OP.mult)

                # transpose evals in tile-groups of (3,3,2); all operands base 0
                vcT = pp.tile([3 * W, 3 * P], f32, tag="vcT")
                widths = [3 * W, 3 * W, 2 * W]
                for j, wdt in enumerate(widths):
                    nc.tensor.transpose(
                        out=vcT[0:wdt, j * P:(j + 1) * P],
                        in_=vc[:, j * 3 * W: j * 3 * W + wdt],
                        identity=identf[:])
                vcT_sb = sp.tile([3 * W, 3 * P], f32, tag="vcTs")
                nc.vector.tensor_copy(out=vcT_sb[:], in_=vcT[:])

                # block-diagonal inverse-DFT: one matmul per tile-group
                o_ps = pp.tile([P, FO], f32, tag="o")
                nc.tensor.matmul(
                    out=o_ps[:, 0:3 * W], lhsT=vcT_sb[0:3 * W, 0:P],
                    rhs=w2r3_sb[:], start=True, stop=True)
                nc.tensor.matmul(
                    out=o_ps[:, 3 * W:6 * W], lhsT=vcT_sb[0:3 * W, P:2 * P],
                    rhs=w2r3_sb[:], start=True, stop=True)
                nc.tensor.matmul(
                    out=o_ps[:, 6 * W:8 * W], lhsT=vcT_sb[0:2 * W, 2 * P:3 * P],
                    rhs=w2r2_sb[:], start=True, stop=True)

                out_sb = sp.tile([P, FO], f32, tag="osb")
                nc.vector.tensor_copy(out=out_sb[:], in_=o_ps[:])
                nc.scalar.dma_start(
                    out=out_v[:, ci * FO:(ci + 1) * FO], in_=out_sb[:])

    nc.compile()
    return nc


def kernel(x: np.ndarray, shuffle_vector: np.ndarray) -> np.ndarray:
    global _cached
    x = np.asarray(x)
    assert x.shape == (B, K), x.shape
    x_bf = np.ascontiguousarray(x.astype(ml_dtypes.bfloat16))

    tabs = _tables(shuffle_vector)
    if _cached is None:
        _cached = _build_module()
    nc = _cached

    shards = x_bf.reshape(NCORES, RPC, K)
    in_maps = [
        {"x": np.ascontiguousarray(shards[i]), **tabs}
        for i in range(NCORES)
    ]
    res = bass_utils.run_bass_kernel_spmd(nc, in_maps, core_ids=list(range(NCORES)))
    out = np.concatenate([res.results[i]["out"] for i in range(NCORES)], axis=0)
    return np.ascontiguousarray(out).view(np.complex64).reshape(B, M).astype(np.complex128)

